# revision 3
# baseline (speedup 1.0000x reference)
"""Trainium2 Bass kernel for the ExemplarHead classification problem (v2, bf16).

Math: per (task, way), with R the 5x1024 class reps (support+noise),
H = I - (1/5)11^T, G = H R R^T H, the SVD head reduces exactly to
    C = W R,  W = I - lam * (lam I + G)^{-1} H
    logits[q,(w,s)] = (2 q.C - ||q||^2 - ||C||^2) / d
(lam I + G) inverse via one scaled Newton step (residual (I-aK)^4 ~ 8e-4,
below the bf16 noise floor). All 20 (task,way) blocks per core are one
masked block-diagonal 100x100 problem.

v2 changes vs v1 (60.4us):
 - all large matmuls in bf16 (1 PE cycle/col vs fp32's 2x2), psum fp32
 - q arrives pre-transposed from host (qT) -> no PE transposes at all
 - one Newton iteration instead of two
 - single packed output DMA; norm folds stay fp32 for accuracy

Sharding: data-parallel over the 32 tasks -> 4 tasks per NeuronCore x 8.
"""

import numpy as np
import ml_dtypes

import concourse.bass as bass
import concourse.mybir as mybir
import concourse.tile as tile
from concourse import bacc
from concourse.bass_utils import run_bass_kernel_spmd

F32 = mybir.dt.float32
BF16 = mybir.dt.bfloat16
AF = mybir.ActivationFunctionType
ALU = mybir.AluOpType

LAM = 100000.0
GMAX_BOUND = 40000.0            # safe bound on ||G|| (observed max ~2.2e4)
ALPHA = 2.0 / (2.0 * LAM + GMAX_BOUND)

N_CORES = 8
T_FULL, NQ, D = 32, 75, 1024
NW, NS = 5, 5
TPC = T_FULL // N_CORES          # tasks per core = 4
NR = TPC * NW * NS               # R rows per core = 100
NCH = D // 128                   # 8 contraction chunks
NJ = NW * NS                     # 25 (way,shot) pairs per task
CF_COLS = 400                    # fp32 const tile columns
CB_COLS = 276                    # bf16 const tile columns


def _host_consts():
    """Packed constant tiles. cF fp32 [128,475], cB bf16 [128,201].

    cF cols: 0:100 alpha*blockmask, 100:200 alpha*lam*I, 200:300 2I,
             300:400 I, col 400:475 ones row (partition 0)
    cB cols: 0:100 H (block-diag), 100:200 alpha*lam*H, col 200 = -0.5
    """
    H5 = np.eye(NS) - np.ones((NS, NS)) / NS
    H_bd = np.kron(np.eye(TPC * NW), H5).astype(np.float32)       # [100,100]
    blockmask = np.kron(np.eye(TPC * NW), np.ones((NS, NS))).astype(np.float32)
    eye = np.eye(NR, dtype=np.float32)
    cF = np.zeros((128, CF_COLS), dtype=np.float32)
    cF[0:NR, 0:NR] = ALPHA * blockmask
    cF[0:NR, NR:2 * NR] = ALPHA * LAM * eye
    cF[0:NR, 2 * NR:3 * NR] = 2.0 * eye
    cF[0:NR, 3 * NR:4 * NR] = eye
    cB = np.zeros((128, CB_COLS), dtype=np.float32)
    cB[0:NR, 0:NR] = H_bd
    cB[0:NR, NR:2 * NR] = ALPHA * LAM * H_bd
    cB[:, 200] = -0.5
    cB[0, 201:201 + NQ] = 1.0
    return cF, cB.astype(ml_dtypes.bfloat16)


def build_nc():
    nc = bacc.Bacc("TRN2")

    qt_d = nc.declare_dram_parameter("qt", [NCH, 128, TPC * NQ], BF16,
                                     isOutput=False)
    qn_d = nc.declare_dram_parameter("qn", [NQ, TPC * D], BF16, isOutput=False)
    sn_d = nc.declare_dram_parameter("sn", [NR, D], F32, isOutput=False)
    nz_d = nc.declare_dram_parameter("nz", [NR, D], F32, isOutput=False)
    cF_d = nc.declare_dram_parameter("cF", [128, CF_COLS], F32, isOutput=False)
    cB_d = nc.declare_dram_parameter("cB", [128, CB_COLS], BF16, isOutput=False)
    out_d = nc.declare_dram_parameter("out", [NQ, TPC * NJ], F32, isOutput=True)

    with tile.TileContext(nc) as tc:
        with (
            tc.tile_pool(name="consts", bufs=1) as consts,
            tc.tile_pool(name="sb", bufs=1) as sb,
            tc.tile_pool(name="scr", bufs=2) as scr,
            tc.tile_pool(name="pipe", bufs=3, space="PSUM") as pipe,
            tc.tile_pool(name="gp", bufs=1, space="PSUM") as gp,
            tc.tile_pool(name="cnp", bufs=1, space="PSUM") as cnp,
            tc.tile_pool(name="qcp", bufs=2, space="PSUM") as qcp,
        ):
            # ---- R inputs first on the SP HWDGE ring (gate the PE) ----
            sn_sb = sb.tile([NR, D], F32)
            nz_sb = sb.tile([NR, D], F32)
            HD = D // 2
            for h in range(2):
                sl = slice(h * HD, (h + 1) * HD)
                nc.sync.dma_start(out=sn_sb[:, sl], in_=sn_d[:, sl])
                nc.sync.dma_start(out=nz_sb[:, sl], in_=nz_d[:, sl])
            cB = consts.tile([128, CB_COLS], BF16)
            nc.sync.dma_start(out=cB, in_=cB_d[:])
            cF = consts.tile([128, CF_COLS], F32)
            nc.sync.dma_start(out=cF, in_=cF_d[:])
            c_amask = cF[0:NR, 0:NR]
            c_alI = cF[0:NR, NR:2 * NR]
            c_2I = cF[0:NR, 2 * NR:3 * NR]
            c_I = cF[0:NR, 3 * NR:4 * NR]
            c_Hb = cB[0:NR, 0:NR]
            c_alHb = cB[0:NR, NR:2 * NR]
            neghb = cB[:, 200:201]
            ones75b = cB[0:1, 201:201 + NQ]

            # early DVE touch so later DVE ops don't re-wait the const sems
            warm = sb.tile([1, 2], F32)
            nc.vector.tensor_copy(warm[0:1, 0:1], cF[0:1, 0:1])
            nc.vector.tensor_copy(warm[0:1, 1:2], cB[0:1, 0:1])

            # ---- q loads on the second (Activation) HWDGE ring ----
            qtb = sb.tile([128, NCH * TPC * NQ], BF16)
            for k in range(NCH):
                nc.scalar.dma_start(out=qtb[:, k * 300:(k + 1) * 300],
                                    in_=qt_d[k])
            qn_nat = sb.tile([NQ, TPC * D], BF16)
            nc.scalar.dma_start(out=qn_nat, in_=qn_d[:])

            # ---- R = support + noise on DVE (fuses the bf16 cast) ----
            rb = sb.tile([NR, D], BF16)
            for h in range(2):
                sl = slice(h * HD, (h + 1) * HD)
                nc.vector.tensor_add(rb[:, sl], sn_sb[:, sl], nz_sb[:, sl])

            # ---- ||q||^2 per task (scalar engine, overlaps PE phase) ----
            qnorm = sb.tile([NQ, TPC], F32)
            qbias = sb.tile([NQ, TPC], F32)
            for t in range(TPC):
                sq_scr = scr.tile([NQ, D], BF16, tag="sq")
                nc.scalar.activation(sq_scr, qn_nat[:, t * D:(t + 1) * D],
                                     AF.Square, accum_out=qnorm[:, t:t + 1])
            nc.scalar.activation(qbias, qnorm, AF.Copy, scale=-1.0 / D)

            # ---- RcT = (H R)^T by chunks (bf16) ----
            rctb = sb.tile([128, NCH * NR], BF16)
            for p in range(2):
                rct_ps = pipe.tile([128, 4 * NR], F32, space="PSUM", tag="pp")
                for kk in range(4):
                    k = 4 * p + kk
                    nc.tensor.matmul(rct_ps[:, kk * NR:(kk + 1) * NR],
                                     lhsT=rb[:, k * 128:(k + 1) * 128],
                                     rhs=c_Hb, start=True, stop=True)
                nc.vector.tensor_copy(rctb[:, p * 4 * NR:(p + 1) * 4 * NR],
                                      rct_ps)

            # ---- G = sum_k RcT_k^T RcT_k ----
            g_ps = gp.tile([NR, NR], F32, space="PSUM")
            for k in range(NCH):
                rct_k = rctb[:, k * NR:(k + 1) * NR]
                nc.tensor.matmul(g_ps, lhsT=rct_k, rhs=rct_k,
                                 start=(k == 0), stop=(k == NCH - 1))

            # ---- K_alpha, one Newton step, W^T ----
            gm_f = sb.tile([NR, NR], F32)
            nc.vector.tensor_mul(gm_f, g_ps, c_amask)
            ka_f = sb.tile([NR, NR], F32)
            nc.vector.tensor_add(ka_f, gm_f, c_alI)
            ka_b = sb.tile([NR, NR], BF16)
            nc.scalar.copy(ka_b, ka_f)                     # ACT, overlaps DVE
            y1_b = sb.tile([NR, NR], BF16)
            nc.vector.tensor_sub(y1_b, c_2I, ka_f)         # Y1 = 2I - Ka
            p_ps = pipe.tile([NR, NR], F32, space="PSUM", tag="pp")
            nc.tensor.matmul(p_ps, lhsT=ka_b, rhs=y1_b, start=True, stop=True)
            qq_b = sb.tile([NR, NR], BF16)
            nc.vector.tensor_sub(qq_b, c_2I, p_ps)         # 2I - Ka Y1
            y2_ps = pipe.tile([NR, NR], F32, space="PSUM", tag="pp")
            nc.tensor.matmul(y2_ps, lhsT=y1_b, rhs=qq_b, start=True, stop=True)
            y2_b = sb.tile([NR, NR], BF16)
            nc.scalar.copy(y2_b, y2_ps)
            hy_ps = pipe.tile([NR, NR], F32, space="PSUM", tag="pp")
            nc.tensor.matmul(hy_ps, lhsT=c_alHb, rhs=y2_b, start=True,
                             stop=True)
            wt_b = sb.tile([NR, NR], BF16)
            nc.vector.tensor_sub(wt_b, c_I, hy_ps)         # W^T = I - alH Y

            # ---- C^T chunks (bf16) + squares for ||C||^2 ----
            ctb = sb.tile([128, NCH * NR], BF16)
            csqb = sb.tile([128, NCH * NR], BF16)
            for p in range(2):
                ct_ps = pipe.tile([128, 4 * NR], F32, space="PSUM", tag="pp")
                for kk in range(4):
                    k = 4 * p + kk
                    nc.tensor.matmul(ct_ps[:, kk * NR:(kk + 1) * NR],
                                     lhsT=rb[:, k * 128:(k + 1) * 128],
                                     rhs=wt_b, start=True, stop=True)
                sl = slice(p * 4 * NR, (p + 1) * 4 * NR)
                nc.vector.tensor_copy(ctb[:, sl], ct_ps)
                nc.scalar.activation(csqb[:, sl], ct_ps, AF.Square)

            # ---- cn row: [1,100] = sum_d -0.5 * C^T(d,j)^2 (fp32 result) ----
            cn_ps = cnp.tile([1, NR], F32, space="PSUM")
            for k in range(NCH):
                nc.tensor.matmul(cn_ps, lhsT=neghb,
                                 rhs=csqb[:, k * NR:(k + 1) * NR],
                                 start=(k == 0), stop=(k == NCH - 1))
            cn_f = sb.tile([1, NR], F32)
            nc.scalar.copy(cn_f, cn_ps)
            cnh_b = sb.tile([1, NR], BF16)
            nc.scalar.copy(cnh_b, cn_ps)
            cnh_f = sb.tile([1, NR], F32)
            nc.scalar.copy(cnh_f, cnh_b)
            cnr_b = sb.tile([1, NR], BF16)
            nc.vector.tensor_sub(cnr_b, cn_f, cnh_f)

            # ---- QC per task + fp32 rank-1 cn fold + fused epilogue ----
            out_sb = sb.tile([NQ, TPC * NJ], F32)
            for t in range(TPC):
                qc_ps = qcp.tile([NQ, NJ], F32, space="PSUM", tag="qc",
                                 name=f"qc{t}")
                for k in range(NCH):
                    lhs = qtb[:, k * 300 + t * NQ:k * 300 + (t + 1) * NQ]
                    rhs = ctb[:, k * NR + t * NJ:k * NR + t * NJ + NJ]
                    nc.tensor.matmul(qc_ps, lhsT=lhs, rhs=rhs,
                                     start=(k == 0), stop=False)
                nc.tensor.matmul(qc_ps, lhsT=ones75b,
                                 rhs=cnh_b[0:1, t * NJ:(t + 1) * NJ],
                                 start=False, stop=False)
                nc.tensor.matmul(qc_ps, lhsT=ones75b,
                                 rhs=cnr_b[0:1, t * NJ:(t + 1) * NJ],
                                 start=False, stop=True)
                # logits = (2/D)*psum + (-qn/D), one dual-op DVE instr
                nc.vector.tensor_scalar(out_sb[:, t * NJ:(t + 1) * NJ],
                                        qc_ps, 2.0 / D, qbias[:, t:t + 1],
                                        ALU.mult, ALU.add)
            nc.sync.dma_start(out=out_d[:], in_=out_sb)

    nc.finalize()
    return nc


_NC_CACHE = None


def _get_nc():
    global _NC_CACHE
    if _NC_CACHE is None:
        _NC_CACHE = build_nc()
    return _NC_CACHE


def make_in_maps(query, support, noise):
    query = np.asarray(query, dtype=np.float32)
    support = np.asarray(support, dtype=np.float32)
    noise = np.asarray(noise, dtype=np.float32)
    cF, cB = _host_consts()
    in_maps = []
    for c in range(N_CORES):
        ts = slice(c * TPC, (c + 1) * TPC)
        qc = query[ts]                                   # (4, 75, 1024)
        qt = np.ascontiguousarray(
            qc.transpose(2, 0, 1).reshape(NCH, 128, TPC * NQ)
        ).astype(ml_dtypes.bfloat16)
        qn = np.ascontiguousarray(
            qc.transpose(1, 0, 2).reshape(NQ, TPC * D)
        ).astype(ml_dtypes.bfloat16)
        in_maps.append({
            "qt": qt,
            "qn": qn,
            "sn": np.ascontiguousarray(support[ts]).reshape(NR, D),
            "nz": np.ascontiguousarray(
                noise[:, ts].transpose(1, 0, 2, 3)).reshape(NR, D),
            "cF": cF,
            "cB": cB,
        })
    return in_maps


def kernel(query, support, noise, support_labels=None, n_way=None, n_shot=None,
           **_unused):
    nc = _get_nc()
    in_maps = make_in_maps(query, support, noise)
    res = run_bass_kernel_spmd(nc, in_maps, list(range(N_CORES)))
    outs = [np.asarray(r["out"]).reshape(NQ, TPC, NJ).transpose(1, 0, 2)
            for r in res.results]
    full = np.concatenate(outs, axis=0)            # (32, 75, 25)
    return full.reshape(T_FULL, NQ, NW, NS).astype(np.float32)
